# revision 51
# baseline (speedup 1.0000x reference)
"""Trainium2 Bass kernel for the DinMod LSTM+CfC (NCP) recurrent network.

Strategy:
  - Data-parallel over 8 NeuronCores: batch 64 -> 8 per core, weights replicated.
  - Phase A (parallel): fc1 projection feats = x @ fc1_w.T + b, then the
    time-invariant input projections of the LSTM cell and CfC layer 0 are
    precomputed for all T steps as big matmuls (transposed layout: feature
    dim on partitions, (t, b) on the free dim).
  - Phase B (sequential scan over T=512): tiny per-step recurrent cell with a
    minimized critical path:
      * precomputed per-step terms are injected into PSUM via identity
        matmuls (off the critical path) and the recurrent matmuls accumulate
        on top (start=False),
      * sigmoid(x) for CfC gating is computed as 0.5*tanh(0.5x)+0.5 with the
        0.5 folded into weights, so each CfC layer needs a single Tanh
        activation op,
      * elementwise gate algebra is packed into quadrant-aligned "stacked"
        tiles so one vector op covers two gates,
      * linear combinations (c = m1+m2, d/s terms) are computed on the tensor
        engine with small constant matrices.
  - All state blocks live at SBUF partition bases in {0, 32, 64, 96} to
    satisfy the quadrant addressing constraints of the compute engines.

Layout conventions (partition ranges):
  h_g (gapped state, 67p): inter 0:18 | cmd 32:44 | motor 64:67, gaps = 0
  P_Y/Y (97p):  fg 0:33 | ig 64:97
  P_AO (97p):   og 0:33 | ia 64:97
  X (97p):      c 0:33  | tanh(ia) 64:97
  P_l/F_l (CfC layer l, k outputs): f1 0:k | f2 k:2k | t 64:64+k
  DS_l: d=0.5*(f2-f1) 0:k | s=0.5*(f1+f2) 64:64+k
"""

import numpy as np

import concourse.bass as bass
import concourse.mybir as mybir
from concourse import bacc
from concourse.tile import TileContext
from concourse.tile_rust import add_dep_helper
from concourse.bass_utils import run_bass_kernel_spmd

IN_DIM, LATENT = 512, 256
INTER, COMMAND, MOTOR = 18, 12, 3
STATE = INTER + COMMAND + MOTOR  # 33
B, T_FULL, N_CORES = 64, 512, 8
BS = B // N_CORES  # 8

F32 = mybir.dt.float32
AF = mybir.ActivationFunctionType


def _gpos(j):
    """state index (0..32) -> gapped partition position."""
    if j < INTER:
        return j
    if j < INTER + COMMAND:
        return 32 + (j - INTER)
    return 64 + (j - INTER - COMMAND)


def prep_weights(inp):
    """Fold/transpose all model weights into device layouts. numpy f32."""
    g = {k: np.asarray(v, np.float32) for k, v in inp.items()}
    w = {}
    w["fc1T"] = np.ascontiguousarray(g["fc1_w"].T)             # (512, 256)
    fb = np.zeros((128, 2), np.float32)
    fb[:, 0] = g["fc1_b"][:128]
    fb[:, 1] = g["fc1_b"][128:]
    w["fc1b"] = fb

    wi, bi, wh = g["lstm_wi"], g["lstm_bi"], g["lstm_wh"]
    ia, ig, fg, og = (slice(0, 33), slice(33, 66), slice(66, 99), slice(99, 132))

    def pack97(rows_lo, rows_hi, src, axis_cols):
        """Build (axis_cols, 97) matrix: cols 0:33 <- src[rows_lo], 64:97 <- src[rows_hi]."""
        m = np.zeros((axis_cols, 97), np.float32)
        m[:, 0:33] = src[rows_lo].T
        m[:, 64:97] = src[rows_hi].T
        return m

    w["wiT_Y"] = pack97(fg, ig, wi, LATENT)                     # (256, 97)
    w["wiT_AO"] = pack97(og, ia, wi, LATENT)
    bY = np.zeros((97, 1), np.float32)
    bY[0:33, 0] = bi[fg] + 1.0
    bY[64:97, 0] = bi[ig]
    w["biasY"] = bY
    bAO = np.zeros((97, 1), np.float32)
    bAO[0:33, 0] = bi[og]
    bAO[64:97, 0] = bi[ia]
    w["biasAO"] = bAO

    # LSTM recurrent weights.  h_state = concat(hl_0, hl_1, hl_2) and each
    # hl_l = s_l + p_l with s_l = Ms_l @ [f1;f2].  We never materialize h:
    # consumers contract directly against [F_l (with p packed at rows 96:)],
    # with lhsT rows 0:2k = Ms_l.T @ W_block, rows 96:96+k = W_block.
    # hl_l = 0.5*(f1 + f2 + pt_l) where pt_l = t*(f2-f1) sits at rows 96: of
    # the F_l tile; consumers contract F_l directly with [0.5W; 0.5W; 0.5W].
    ks = [INTER, COMMAND, MOTOR]
    koff = [0, INTER, INTER + COMMAND]
    for bi_, k in enumerate(ks):
        for pname, lo_sl, hi_sl in (("Y", fg, ig), ("AO", og, ia)):
            blk = np.zeros((k, 97), np.float32)   # raw W rows for this state block
            for j in range(k):
                blk[j, 0:33] = wh[lo_sl, koff[bi_] + j]
                blk[j, 64:97] = wh[hi_sl, koff[bi_] + j]
            comb = np.zeros((96 + k, 97), np.float32)
            comb[0:k, :] = 0.5 * blk
            comb[32:32 + k, :] = 0.5 * blk
            comb[96:96 + k, :] = 0.5 * blk
            w[f"wh{pname}{bi_}"] = comb

    # CfC layers
    dims = [(LATENT, INTER), (INTER, COMMAND), (COMMAND, MOTOR)]
    for l, (p_l, k) in enumerate(dims):
        w1m = g[f"ff1w{l}"] * g[f"mask{l}"]
        w2m = g[f"ff2w{l}"] * g[f"mask{l}"]
        wab = 0.5 * (g[f"taw{l}"] + g[f"tbw{l}"])
        bti = 0.5 * (g[f"tab{l}"] + g[f"tbb{l}"])
        nrows = 64 + k  # gate rows: f1 0:k | f2 32:32+k | t 64:64+k
        bias = np.zeros((nrows, 1), np.float32)
        bias[0:k, 0] = g[f"ff1b{l}"]
        bias[32:32 + k, 0] = g[f"ff2b{l}"]
        bias[64:64 + k, 0] = bti
        w[f"bias{l}"] = bias

        def pack_cols(col_sel, in_rows, row_map):
            # gate-row layout (quadrant per role): f1 0:k | f2 32:32+k | t 64:64+k
            m = np.zeros((in_rows, nrows), np.float32)
            for jj, rr in row_map:
                m[rr, 0:k] = w1m[:, col_sel][:, jj]
                m[rr, 32:32 + k] = w2m[:, col_sel][:, jj]
                m[rr, 64:64 + k] = wab[:, col_sel][:, jj]
            return m

        if l == 0:
            # input part (from feats, 256) and recurrent part (inter slice)
            w["W0inT"] = pack_cols(slice(0, LATENT), LATENT,
                                   [(jj, jj) for jj in range(LATENT)])
            w["W0recT"] = pack_cols(slice(LATENT, LATENT + k), INTER,
                                    [(jj, jj) for jj in range(INTER)])
        elif l == 1:
            # input = hl0 = 0.5*(f1 + f2 + pt0), contracted against F0ext
            w1in = pack_cols(slice(0, INTER), INTER,
                             [(jj, jj) for jj in range(INTER)])      # (18, 76)
            comb = np.zeros((96 + INTER, nrows), np.float32)
            comb[0:INTER, :] = 0.5 * w1in
            comb[32:32 + INTER, :] = 0.5 * w1in
            comb[96:96 + INTER, :] = 0.5 * w1in
            w["W1comb"] = comb
            w["W1recT"] = pack_cols(slice(INTER, INTER + k), STATE,
                                    [(jj, INTER + jj) for jj in range(COMMAND)])
        else:
            # input = hl1 = 0.5*(f1 + f2 + pt1), contracted against F1ext
            w2in = pack_cols(slice(0, COMMAND), COMMAND,
                             [(jj, jj) for jj in range(COMMAND)])    # (12, 67)
            comb = np.zeros((96 + COMMAND, nrows), np.float32)
            comb[0:COMMAND, :] = 0.5 * w2in
            comb[32:32 + COMMAND, :] = 0.5 * w2in
            comb[96:96 + COMMAND, :] = 0.5 * w2in
            w["W2comb"] = comb
            w["W2recT"] = pack_cols(slice(COMMAND, COMMAND + k), STATE,
                                    [(jj, STATE - MOTOR + jj) for jj in range(MOTOR)])

    bg0 = np.zeros((64 + INTER, 1), np.float32)
    bg0[0:INTER, 0] = g["ff1b0"]
    bg0[32:32 + INTER, 0] = g["ff2b0"]
    bg0[64:64 + INTER, 0] = 0.5 * (g["tab0"] + g["tbb0"])
    w["biasg0"] = bg0

    # constant matrices for tensor-engine linear combos
    cc = np.zeros((97, 33), np.float32)
    for j in range(33):
        cc[j, j] = 1.0
        cc[64 + j, j] = 1.0
    w["Cc"] = cc
    # s2 = 0.5*(f1 + f2) for the motor output (hl2 = s2 + 0.5*pt2)
    c = np.zeros((32 + MOTOR, MOTOR), np.float32)
    for j in range(MOTOR):
        c[j, j] = 0.5
        c[32 + j, j] = 0.5
    w["C2"] = c
    # d_l = f2 - f1 on the tensor engine (PSUM out, so the p-multiply reads
    # one PSUM operand — dual-SBUF TensorTensor requires equal base partitions)
    for l, k in [(0, INTER), (1, COMMAND), (2, MOTOR)]:
        c = np.zeros((32 + k, k), np.float32)
        for j in range(k):
            c[j, j] = -1.0
            c[32 + j, j] = 1.0
        w[f"Cd{l}"] = c
    i97 = np.zeros((97, 97), np.float32)
    for r in list(range(33)) + list(range(64, 97)):
        i97[r, r] = 1.0
    w["I97"] = i97
    i82 = np.zeros((82, 82), np.float32)
    for r in (list(range(INTER)) + list(range(32, 32 + INTER))
              + list(range(64, 64 + INTER))):
        i82[r, r] = 1.0
    w["I82"] = i82
    return w


# DRAM input specs (name -> shape) besides xt
def _weight_specs(T):
    return {
        "fc1T": (512, 256), "fc1b": (128, 2),
        "wiT_Y": (256, 97), "wiT_AO": (256, 97),
        "biasY": (97, 1), "biasAO": (97, 1),
        "whY0": (114, 97), "whY1": (108, 97), "whY2": (99, 97),
        "whAO0": (114, 97), "whAO1": (108, 97), "whAO2": (99, 97),
        "W0inT": (256, 82), "W0recT": (18, 82), "biasg0": (82, 1),
        "W1comb": (114, 76), "W1recT": (33, 76), "bias1": (76, 1),
        "W2comb": (108, 67), "W2recT": (33, 67), "bias2": (67, 1),
        "Cc": (97, 33), "C2": (35, 3),
        "Cd0": (50, 18), "Cd1": (44, 12), "Cd2": (35, 3),
        "I97": (97, 97), "I82": (82, 82),
    }


def build_program(T=T_FULL, opts=()):
    """Build the Bass program for one core: xt (512, T*BS) -> out (3, T*BS)."""
    opts = set(opts)
    NF = T * BS
    nc = bacc.Bacc("TRN2")
    xt_d = nc.dram_tensor("xt", [IN_DIM, NF], F32, kind="ExternalInput")
    wd = {}
    for nm, shp in _weight_specs(T).items():
        wd[nm] = nc.dram_tensor(nm, list(shp), F32, kind="ExternalInput")
    out_d = nc.dram_tensor("out", [MOTOR, NF], F32, kind="ExternalOutput")

    NCH = NF // 512 if NF >= 512 else 1   # free-dim chunks for phase A
    CH = min(512, NF)

    with TileContext(nc) as tc:
        with tc.tile_pool(name="wpool", bufs=1) as wp, \
             tc.tile_pool(name="data", bufs=1) as dp:
            # ---- load weights (tensors with >128 rows are partition-chunked) ----
            sb = {}
            for nm, shp in _weight_specs(T).items():
                rows, cols = shp
                if rows > 128:
                    nch = (rows + 127) // 128
                    t = wp.tile([128, nch, cols], F32, tag=f"w_{nm}")
                    # single DMA per tensor so consumers wait on one queue only
                    nc.sync.dma_start(
                        out=t, in_=wd[nm].rearrange("(c p) n -> p c n", p=128))
                else:
                    t = wp.tile([rows, cols], F32, tag=f"w_{nm}")
                    nc.sync.dma_start(out=t, in_=wd[nm][:, :])
                sb[nm] = t

            # ---- load x (transposed on host): 4 chunks of 128 partitions ----
            xt_sb = dp.tile([128, 4, NF], F32)
            nc.sync.dma_start(out=xt_sb,
                              in_=xt_d.rearrange("(c p) n -> p c n", p=128))

            feats = dp.tile([128, 2, NF], F32)
            zinY = dp.tile([97, NF], F32)
            zinAO = dp.tile([97, NF], F32)
            g0in = dp.tile([82, NF], F32)
            out_sb = dp.tile([MOTOR, NF], F32)

            # ---- Phase A: big parallel matmuls ----
            with tc.tile_pool(name="pa", bufs=2, space="PSUM") as pa:
                # Wait-absorbers: the fused LDWEIGHTS+MATMUL can carry only one
                # semaphore wait, so have the PE observe every DMA-loaded tile
                # once via a 1x1 dummy matmul (one producer each, disjoint
                # PSUM columns so no write-ordering self-wait is added);
                # real matmuls then need at most one new wait.
                srcs = list(sb.values()) + [xt_sb]
                absorb = pa.tile([1, len(srcs) + 3], F32, tag="absorb")
                for j, t_ in enumerate(srcs):
                    a2 = t_[0:1, 0, 0:1] if len(t_.shape) == 3 else t_[0:1, 0:1]
                    nc.tensor.matmul(absorb[:, j:j + 1], a2, a2, start=True, stop=True)
                for m in range(2):
                    for n in range(NCH):
                        ps = pa.tile([128, CH], F32)
                        for k in range(4):
                            nc.tensor.matmul(
                                ps,
                                sb["fc1T"][:, k, 128 * m:128 * (m + 1)],
                                xt_sb[:, k, n * CH:(n + 1) * CH],
                                start=(k == 0), stop=(k == 3),
                            )
                        nc.scalar.activation(
                            feats[:, m, n * CH:(n + 1) * CH], ps,
                            AF.Identity, bias=sb["fc1b"][:, m:m + 1],
                        )
                for tgt, lhs, bias_nm, rows in (
                    (zinY, "wiT_Y", "biasY", 97),
                    (zinAO, "wiT_AO", "biasAO", 97),
                    (g0in, "W0inT", "biasg0", 82),
                ):
                    for n in range(NCH):
                        ps = pa.tile([128, CH], F32)
                        for kk in range(2):
                            nc.tensor.matmul(
                                ps[0:rows, :],
                                sb[lhs][:, kk, :],
                                feats[:, kk, n * CH:(n + 1) * CH],
                                start=(kk == 0), stop=(kk == 1),
                            )
                        nc.scalar.activation(
                            tgt[:, n * CH:(n + 1) * CH], ps[0:rows, :],
                            AF.Identity, bias=sb[bias_nm][:, 0:1],
                        )
                # absorb the first zin-chunk ACT waits so the t=0 injects
                # carry a single semaphore wait
                zin_anchor = None
                for j, tgt in enumerate((zinY, zinAO, g0in)):
                    a2 = tgt[0:1, NF - 1:NF]   # last chunk -> max ACT sem value
                    zin_anchor = nc.tensor.matmul(
                        absorb[:, len(srcs) + j:len(srcs) + j + 1],
                        a2, a2, start=True, stop=True)

            # ---- Phase B: the scan ----
            with tc.tile_pool(name="st", bufs=1) as stp, \
                 tc.tile_pool(name="sc", bufs=3) as scp, \
                 tc.tile_pool(name="pY", bufs=1, space="PSUM") as pY, \
                 tc.tile_pool(name="pAO", bufs=1, space="PSUM") as pAO, \
                 tc.tile_pool(name="pC", bufs=1, space="PSUM") as pC, \
                 tc.tile_pool(name="p0", bufs=1, space="PSUM") as p0p, \
                 tc.tile_pool(name="p1", bufs=1, space="PSUM") as p1p, \
                 tc.tile_pool(name="p2", bufs=1, space="PSUM") as p2p, \
                 tc.tile_pool(name="pDS", bufs=1, space="PSUM") as pDS:

                # ping-pong state tiles; rows outside the written blocks are
                # memset to zero once and never rewritten
                xA = stp.tile([97, BS], F32, tag="xA")
                xB = stp.tile([97, BS], F32, tag="xB")
                f0A = stp.tile([96 + INTER, BS], F32, tag="f0A")
                f0B = stp.tile([96 + INTER, BS], F32, tag="f0B")
                f1A = stp.tile([96 + COMMAND, BS], F32, tag="f1A")
                f1B = stp.tile([96 + COMMAND, BS], F32, tag="f1B")
                f2A = stp.tile([96 + MOTOR, BS], F32, tag="f2A")
                f2B = stp.tile([96 + MOTOR, BS], F32, tag="f2B")
                for t_ in (xA, xB, f0A, f0B, f1A, f1B, f2A, f2B):
                    nc.vector.memset(t_, 0.0)
                x_pair = (xA, xB)
                f_pairs = ((f0A, f0B), (f1A, f1B), (f2A, f2B))
                prev_anchor = zin_anchor  # last Cd matmul of the previous step

                scan_reps = 1
                for o in opts:
                    if isinstance(o, str) and o.startswith("reps"):
                        scan_reps = int(o[4:])
                steps = [(rep, t) for rep in range(scan_reps) for t in range(T)]
                for rep, t in steps:
                    x_cur = x_pair[t % 2]
                    x_next = x_pair[(t + 1) % 2]
                    F0p, F1p, F2p = (fp[(t + 1) % 2] for fp in f_pairs)   # prev step
                    F0, F1, F2 = (fp[t % 2] for fp in f_pairs)            # this step
                    c0, c1 = t * BS, (t + 1) * BS

                    # --- LSTM gates: zin inject + recurrent part contracted
                    #     against the previous step's F tiles (h never formed) ---
                    PY = pY.tile([97, BS], F32)
                    PA = pAO.tile([97, BS], F32)
                    last = (t == 0)
                    miY = nc.tensor.matmul(PY, sb["I97"], zinY[:, c0:c1], start=True, stop=last)
                    miA = nc.tensor.matmul(PA, sb["I97"], zinAO[:, c0:c1], start=True, stop=last)
                    # keep injects behind the previous step's last Cd matmul in
                    # PE order so their WAR-on-ACT wait is already subsumed
                    # (the fused LDWEIGHTS can carry only one sem wait)
                    if prev_anchor is not None:
                        add_dep_helper(miY.ins, prev_anchor.ins, sync=False,
                                       reason="inject after prev Cd (wait budget)")
                        add_dep_helper(miA.ins, prev_anchor.ins, sync=False,
                                       reason="inject after prev Cd (wait budget)")
                    if t > 0:
                        for i, (Fprev, kb) in enumerate(((F0p, INTER), (F1p, COMMAND), (F2p, MOTOR))):
                            lastb = (i == 2)
                            nc.tensor.matmul(PY, sb[f"whY{i}"], Fprev[:, :],
                                             start=False, stop=lastb)
                            nc.tensor.matmul(PA, sb[f"whAO{i}"], Fprev[:, :],
                                             start=False, stop=lastb)

                    Y = scp.tile([97, BS], F32, tag="Y")
                    nc.scalar.activation(Y, PY, AF.Sigmoid)               # sig(fg)|sig(ig)
                    nc.scalar.activation(x_cur[64:97, :], PA[64:97, :], AF.Tanh)  # tanh(ia)
                    O = scp.tile([33, BS], F32, tag="O")
                    nc.scalar.activation(O, PA[0:33, :], AF.Sigmoid)      # sig(og)

                    S = scp.tile([97, BS], F32, tag="S")
                    nc.vector.tensor_mul(S, x_cur, Y)                     # c*sfg | T_ia*sig
                    PCt = pC.tile([33, BS], F32)
                    nc.tensor.matmul(PCt, sb["Cc"], S, start=True, stop=True)  # c_new
                    Tc = scp.tile([33, BS], F32, tag="Tc")
                    nc.scalar.activation(Tc, PCt, AF.Tanh)
                    nc.scalar.copy(x_next[0:33, :], PCt)                  # carry c
                    hl = scp.tile([33, BS], F32, tag="hl")
                    nc.vector.tensor_mul(hl, Tc, O)                       # h_lstm

                    # --- CfC layer 0 ---
                    P0 = p0p.tile([82, BS], F32)
                    mi0 = nc.tensor.matmul(P0, sb["I82"], g0in[:, c0:c1], start=True, stop=False)
                    if prev_anchor is not None:
                        add_dep_helper(mi0.ins, prev_anchor.ins, sync=False,
                                       reason="inject after prev Cd (wait budget)")
                    nc.tensor.matmul(P0, sb["W0recT"], hl[0:18, :], start=False, stop=True)
                    nc.scalar.activation(F0[0:82, :], P0, AF.Tanh)
                    D0 = pDS.tile([INTER, BS], F32, tag="DSd")
                    nc.tensor.matmul(D0, sb["Cd0"], F0[0:32 + INTER, :], start=True, stop=True)
                    nc.vector.tensor_mul(F0[96:96 + INTER, :], F0[64:64 + INTER, :], D0)

                    # --- CfC layer 1 ---
                    P1 = p1p.tile([76, BS], F32)
                    nc.tensor.matmul(P1, sb["W1recT"], hl[0:33, :], start=True, stop=False)
                    nc.tensor.matmul(P1, sb["W1comb"], F0[:, :], start=False, stop=True)
                    nc.scalar.activation(F1[0:76, :], P1, AF.Tanh, bias=sb["bias1"][:, 0:1])
                    D1 = pDS.tile([COMMAND, BS], F32, tag="DSd")
                    nc.tensor.matmul(D1, sb["Cd1"], F1[0:32 + COMMAND, :], start=True, stop=True)
                    nc.vector.tensor_mul(F1[96:96 + COMMAND, :], F1[64:64 + COMMAND, :], D1)

                    # --- CfC layer 2 ---
                    P2 = p2p.tile([67, BS], F32)
                    nc.tensor.matmul(P2, sb["W2recT"], hl[0:33, :], start=True, stop=False)
                    nc.tensor.matmul(P2, sb["W2comb"], F1[:, :], start=False, stop=True)
                    nc.scalar.activation(F2[0:67, :], P2, AF.Tanh, bias=sb["bias2"][:, 0:1])
                    D2 = pDS.tile([MOTOR, BS], F32, tag="DSd")
                    prev_anchor = nc.tensor.matmul(
                        D2, sb["Cd2"], F2[0:32 + MOTOR, :], start=True, stop=True)
                    nc.vector.tensor_mul(F2[96:96 + MOTOR, :], F2[64:64 + MOTOR, :], D2)
                    # motor output hl2 = s2 + 0.5*pt2 (off the critical chain)
                    DS2 = pDS.tile([MOTOR, BS], F32, tag="DS")
                    nc.tensor.matmul(DS2, sb["C2"], F2[0:32 + MOTOR, :], start=True, stop=True)
                    nc.vector.scalar_tensor_tensor(
                        out_sb[:, c0:c1], F2[96:96 + MOTOR, :], 0.5, DS2,
                        mybir.AluOpType.mult, mybir.AluOpType.add)

            nc.sync.dma_start(out=out_d[:, :], in_=out_sb[:, :])
    nc.compile()   # bacc passes: split multi-waits into event semaphores etc.
    return nc


def host_prep(inputs, T=T_FULL):
    """Shard + transpose x per core; fold weights (shared)."""
    x = np.asarray(inputs["x"], np.float32)
    w = prep_weights(inputs)
    in_maps = []
    for i in range(N_CORES):
        xs = x[i * BS:(i + 1) * BS, :T, :]                  # (BS, T, 512)
        xt = np.ascontiguousarray(xs.transpose(2, 1, 0).reshape(IN_DIM, T * BS))
        m = {"xt": xt}
        m.update(w)
        in_maps.append(m)
    return in_maps


def gather_output(results, T=T_FULL):
    outs = []
    for i in range(N_CORES):
        o = np.asarray(results[i]["out"])                   # (3, T*BS)
        outs.append(o.reshape(MOTOR, T, BS).transpose(2, 1, 0))  # (BS, T, 3)
    return np.concatenate(outs, axis=0)


_PROGRAM_CACHE = {}


def kernel(**inputs):
    T = T_FULL
    if T not in _PROGRAM_CACHE:
        _PROGRAM_CACHE[T] = build_program(T)
    nc = _PROGRAM_CACHE[T]
    in_maps = host_prep(inputs, T)
    res = run_bass_kernel_spmd(nc, in_maps, list(range(N_CORES)))
    return gather_output(res.results, T)


# revision 52
# speedup vs baseline: 1.9663x; 1.9663x over previous
"""Trainium2 Bass kernel for the DinMod LSTM+CfC (NCP) recurrent network.

Strategy:
  - Data-parallel over 8 NeuronCores: batch 64 -> 8 per core, weights replicated.
  - Phase A (parallel): fc1 projection feats = x @ fc1_w.T + b, then the
    time-invariant input projections of the LSTM cell and CfC layer 0 are
    precomputed for all T steps as big matmuls (transposed layout: feature
    dim on partitions, (t, b) on the free dim).
  - Phase B (sequential scan over T=512): tiny per-step recurrent cell with a
    minimized critical path:
      * precomputed per-step terms are injected into PSUM via identity
        matmuls (off the critical path) and the recurrent matmuls accumulate
        on top (start=False),
      * sigmoid for the CfC gate is 0.5*tanh(0.5x)+0.5 with the 0.5 folded
        into weights, so each CfC layer needs a single Tanh activation op,
      * elementwise gate algebra is packed into quadrant-aligned "stacked"
        tiles so one vector op covers two gates,
      * the CfC state hl = 0.5*(f1 + f2 + t*(f2-f1)) is never materialized:
        t*(f2-f1) is written into spare rows of the F tile and every
        consumer contracts F directly with host-precomposed weights.
  - All operand blocks live at SBUF partition bases in {0, 32, 64, 96}
    (quadrant addressing); dual-SBUF vector ops use equal bases, f2-f1 is
    built on the tensor engine (PSUM) so the p-multiply is mixed-space.
  - Every compute instruction carries at most one new semaphore wait
    (hardware limit): absorber 1x1 matmuls observe each DMA once, injects
    are pinned behind the previous step's Cd matmul, and bacc's
    generate_event_semaphores splits anything left.

Layout conventions (partition ranges):
  P_Y/Y (97p):  sig(fg) 0:33 | sig(ig) 64:97
  P_AO (97p):   og 0:33 | ia 64:97
  X (97p):      c 0:33  | tanh(ia) 64:97
  P_l/F_l (CfC layer l, k outputs): f1 0:k | f2 32:32+k | t 64:64+k | pt 96:96+k
"""

import numpy as np

import concourse.bass as bass
import concourse.mybir as mybir
from concourse import bacc
from concourse.tile import TileContext
from concourse.tile_rust import add_dep_helper
from concourse.bass_utils import run_bass_kernel_spmd

IN_DIM, LATENT = 512, 256
INTER, COMMAND, MOTOR = 18, 12, 3
STATE = INTER + COMMAND + MOTOR  # 33
B, T_FULL, N_CORES = 64, 512, 8
BS = B // N_CORES  # 8

F32 = mybir.dt.float32
AF = mybir.ActivationFunctionType


def _gpos(j):
    """state index (0..32) -> gapped partition position."""
    if j < INTER:
        return j
    if j < INTER + COMMAND:
        return 32 + (j - INTER)
    return 64 + (j - INTER - COMMAND)


def prep_weights(inp):
    """Fold/transpose all model weights into device layouts. numpy f32."""
    g = {k: np.asarray(v, np.float32) for k, v in inp.items()}
    w = {}
    w["fc1T"] = np.ascontiguousarray(g["fc1_w"].T)             # (512, 256)
    fb = np.zeros((128, 2), np.float32)
    fb[:, 0] = g["fc1_b"][:128]
    fb[:, 1] = g["fc1_b"][128:]
    w["fc1b"] = fb

    wi, bi, wh = g["lstm_wi"], g["lstm_bi"], g["lstm_wh"]
    ia, ig, fg, og = (slice(0, 33), slice(33, 66), slice(66, 99), slice(99, 132))

    def pack97(rows_lo, rows_hi, src, axis_cols):
        """Build (axis_cols, 97) matrix: cols 0:33 <- src[rows_lo], 64:97 <- src[rows_hi]."""
        m = np.zeros((axis_cols, 97), np.float32)
        m[:, 0:33] = src[rows_lo].T
        m[:, 64:97] = src[rows_hi].T
        return m

    w["wiT_Y"] = pack97(fg, ig, wi, LATENT)                     # (256, 97)
    w["wiT_AO"] = pack97(og, ia, wi, LATENT)
    bY = np.zeros((97, 1), np.float32)
    bY[0:33, 0] = bi[fg] + 1.0
    bY[64:97, 0] = bi[ig]
    w["biasY"] = bY
    bAO = np.zeros((97, 1), np.float32)
    bAO[0:33, 0] = bi[og]
    bAO[64:97, 0] = bi[ia]
    w["biasAO"] = bAO

    # LSTM recurrent weights.  h_state = concat(hl_0, hl_1, hl_2) and each
    # hl_l = s_l + p_l with s_l = Ms_l @ [f1;f2].  We never materialize h:
    # consumers contract directly against [F_l (with p packed at rows 96:)],
    # with lhsT rows 0:2k = Ms_l.T @ W_block, rows 96:96+k = W_block.
    # hl_l = 0.5*(f1 + f2 + pt_l) where pt_l = t*(f2-f1) sits at rows 96: of
    # the F_l tile; consumers contract F_l directly with [0.5W; 0.5W; 0.5W].
    ks = [INTER, COMMAND, MOTOR]
    koff = [0, INTER, INTER + COMMAND]
    for bi_, k in enumerate(ks):
        for pname, lo_sl, hi_sl in (("Y", fg, ig), ("AO", og, ia)):
            blk = np.zeros((k, 97), np.float32)   # raw W rows for this state block
            for j in range(k):
                blk[j, 0:33] = wh[lo_sl, koff[bi_] + j]
                blk[j, 64:97] = wh[hi_sl, koff[bi_] + j]
            comb = np.zeros((96 + k, 97), np.float32)
            comb[0:k, :] = 0.5 * blk
            comb[32:32 + k, :] = 0.5 * blk
            comb[96:96 + k, :] = 0.5 * blk
            w[f"wh{pname}{bi_}"] = comb

    # CfC layers
    dims = [(LATENT, INTER), (INTER, COMMAND), (COMMAND, MOTOR)]
    for l, (p_l, k) in enumerate(dims):
        w1m = g[f"ff1w{l}"] * g[f"mask{l}"]
        w2m = g[f"ff2w{l}"] * g[f"mask{l}"]
        wab = 0.5 * (g[f"taw{l}"] + g[f"tbw{l}"])
        bti = 0.5 * (g[f"tab{l}"] + g[f"tbb{l}"])
        nrows = 64 + k  # gate rows: f1 0:k | f2 32:32+k | t 64:64+k
        bias = np.zeros((nrows, 1), np.float32)
        bias[0:k, 0] = g[f"ff1b{l}"]
        bias[32:32 + k, 0] = g[f"ff2b{l}"]
        bias[64:64 + k, 0] = bti
        w[f"bias{l}"] = bias

        def pack_cols(col_sel, in_rows, row_map):
            # gate-row layout (quadrant per role): f1 0:k | f2 32:32+k | t 64:64+k
            m = np.zeros((in_rows, nrows), np.float32)
            for jj, rr in row_map:
                m[rr, 0:k] = w1m[:, col_sel][:, jj]
                m[rr, 32:32 + k] = w2m[:, col_sel][:, jj]
                m[rr, 64:64 + k] = wab[:, col_sel][:, jj]
            return m

        if l == 0:
            # input part (from feats, 256) and recurrent part (inter slice)
            w["W0inT"] = pack_cols(slice(0, LATENT), LATENT,
                                   [(jj, jj) for jj in range(LATENT)])
            w["W0recT"] = pack_cols(slice(LATENT, LATENT + k), INTER,
                                    [(jj, jj) for jj in range(INTER)])
        elif l == 1:
            # input = hl0 = 0.5*(f1 + f2 + pt0), contracted against F0ext
            w1in = pack_cols(slice(0, INTER), INTER,
                             [(jj, jj) for jj in range(INTER)])      # (18, 76)
            comb = np.zeros((96 + INTER, nrows), np.float32)
            comb[0:INTER, :] = 0.5 * w1in
            comb[32:32 + INTER, :] = 0.5 * w1in
            comb[96:96 + INTER, :] = 0.5 * w1in
            w["W1comb"] = comb
            w["W1recT"] = pack_cols(slice(INTER, INTER + k), STATE,
                                    [(jj, INTER + jj) for jj in range(COMMAND)])
        else:
            # input = hl1 = 0.5*(f1 + f2 + pt1), contracted against F1ext
            w2in = pack_cols(slice(0, COMMAND), COMMAND,
                             [(jj, jj) for jj in range(COMMAND)])    # (12, 67)
            comb = np.zeros((96 + COMMAND, nrows), np.float32)
            comb[0:COMMAND, :] = 0.5 * w2in
            comb[32:32 + COMMAND, :] = 0.5 * w2in
            comb[96:96 + COMMAND, :] = 0.5 * w2in
            w["W2comb"] = comb
            w["W2recT"] = pack_cols(slice(COMMAND, COMMAND + k), STATE,
                                    [(jj, STATE - MOTOR + jj) for jj in range(MOTOR)])

    bg0 = np.zeros((64 + INTER, 1), np.float32)
    bg0[0:INTER, 0] = g["ff1b0"]
    bg0[32:32 + INTER, 0] = g["ff2b0"]
    bg0[64:64 + INTER, 0] = 0.5 * (g["tab0"] + g["tbb0"])
    w["biasg0"] = bg0

    # constant matrices for tensor-engine linear combos
    cc = np.zeros((97, 33), np.float32)
    for j in range(33):
        cc[j, j] = 1.0
        cc[64 + j, j] = 1.0
    w["Cc"] = cc
    # s2 = 0.5*(f1 + f2) for the motor output (hl2 = s2 + 0.5*pt2)
    c = np.zeros((32 + MOTOR, MOTOR), np.float32)
    for j in range(MOTOR):
        c[j, j] = 0.5
        c[32 + j, j] = 0.5
    w["C2"] = c
    # d_l = f2 - f1 on the tensor engine (PSUM out, so the p-multiply reads
    # one PSUM operand — dual-SBUF TensorTensor requires equal base partitions)
    for l, k in [(0, INTER), (1, COMMAND), (2, MOTOR)]:
        c = np.zeros((32 + k, k), np.float32)
        for j in range(k):
            c[j, j] = -1.0
            c[32 + j, j] = 1.0
        w[f"Cd{l}"] = c
    i97 = np.zeros((97, 97), np.float32)
    for r in list(range(33)) + list(range(64, 97)):
        i97[r, r] = 1.0
    w["I97"] = i97
    i82 = np.zeros((82, 82), np.float32)
    for r in (list(range(INTER)) + list(range(32, 32 + INTER))
              + list(range(64, 64 + INTER))):
        i82[r, r] = 1.0
    w["I82"] = i82
    return w


# DRAM input specs (name -> shape) besides xt
def _weight_specs(T):
    return {
        "fc1T": (512, 256), "fc1b": (128, 2),
        "wiT_Y": (256, 97), "wiT_AO": (256, 97),
        "biasY": (97, 1), "biasAO": (97, 1),
        "whY0": (114, 97), "whY1": (108, 97), "whY2": (99, 97),
        "whAO0": (114, 97), "whAO1": (108, 97), "whAO2": (99, 97),
        "W0inT": (256, 82), "W0recT": (18, 82), "biasg0": (82, 1),
        "W1comb": (114, 76), "W1recT": (33, 76), "bias1": (76, 1),
        "W2comb": (108, 67), "W2recT": (33, 67), "bias2": (67, 1),
        "Cc": (97, 33), "C2": (35, 3),
        "Cd0": (50, 18), "Cd1": (44, 12), "Cd2": (35, 3),
        "I97": (97, 97), "I82": (82, 82),
    }


def build_program(T=T_FULL, opts=()):
    """Build the Bass program for one core: xt (512, T*BS) -> out (3, T*BS)."""
    opts = set(opts)
    NF = T * BS
    nc = bacc.Bacc("TRN2")
    xt_d = nc.dram_tensor("xt", [IN_DIM, NF], F32, kind="ExternalInput")
    wd = {}
    for nm, shp in _weight_specs(T).items():
        wd[nm] = nc.dram_tensor(nm, list(shp), F32, kind="ExternalInput")
    out_d = nc.dram_tensor("out", [MOTOR, NF], F32, kind="ExternalOutput")

    NCH = NF // 512 if NF >= 512 else 1   # free-dim chunks for phase A
    CH = min(512, NF)

    with TileContext(nc) as tc:
        with tc.tile_pool(name="wpool", bufs=1) as wp, \
             tc.tile_pool(name="data", bufs=1) as dp:
            # ---- load weights (tensors with >128 rows are partition-chunked) ----
            sb = {}
            for nm, shp in _weight_specs(T).items():
                rows, cols = shp
                if rows > 128:
                    nch = (rows + 127) // 128
                    t = wp.tile([128, nch, cols], F32, tag=f"w_{nm}")
                    # single DMA per tensor so consumers wait on one queue only
                    nc.sync.dma_start(
                        out=t, in_=wd[nm].rearrange("(c p) n -> p c n", p=128))
                else:
                    t = wp.tile([rows, cols], F32, tag=f"w_{nm}")
                    nc.sync.dma_start(out=t, in_=wd[nm][:, :])
                sb[nm] = t

            # ---- load x (transposed on host): 4 chunks of 128 partitions ----
            xt_sb = dp.tile([128, 4, NF], F32)
            nc.sync.dma_start(out=xt_sb,
                              in_=xt_d.rearrange("(c p) n -> p c n", p=128))

            feats = dp.tile([128, 2, NF], F32)
            zinY = dp.tile([97, NF], F32)
            zinAO = dp.tile([97, NF], F32)
            g0in = dp.tile([82, NF], F32)
            out_sb = dp.tile([MOTOR, NF], F32)

            # ---- Phase A: big parallel matmuls ----
            with tc.tile_pool(name="pa", bufs=2, space="PSUM") as pa:
                # Wait-absorbers: the fused LDWEIGHTS+MATMUL can carry only one
                # semaphore wait, so have the PE observe every DMA-loaded tile
                # once via a 1x1 dummy matmul (one producer each, disjoint
                # PSUM columns so no write-ordering self-wait is added);
                # real matmuls then need at most one new wait.
                srcs = list(sb.values()) + [xt_sb]
                absorb = pa.tile([1, len(srcs) + 3], F32, tag="absorb")
                for j, t_ in enumerate(srcs):
                    a2 = t_[0:1, 0, 0:1] if len(t_.shape) == 3 else t_[0:1, 0:1]
                    nc.tensor.matmul(absorb[:, j:j + 1], a2, a2, start=True, stop=True)
                for m in range(2):
                    for n in range(NCH):
                        ps = pa.tile([128, CH], F32)
                        for k in range(4):
                            nc.tensor.matmul(
                                ps,
                                sb["fc1T"][:, k, 128 * m:128 * (m + 1)],
                                xt_sb[:, k, n * CH:(n + 1) * CH],
                                start=(k == 0), stop=(k == 3),
                            )
                        nc.scalar.activation(
                            feats[:, m, n * CH:(n + 1) * CH], ps,
                            AF.Identity, bias=sb["fc1b"][:, m:m + 1],
                        )
                for tgt, lhs, bias_nm, rows in (
                    (zinY, "wiT_Y", "biasY", 97),
                    (zinAO, "wiT_AO", "biasAO", 97),
                    (g0in, "W0inT", "biasg0", 82),
                ):
                    for n in range(NCH):
                        ps = pa.tile([128, CH], F32)
                        for kk in range(2):
                            nc.tensor.matmul(
                                ps[0:rows, :],
                                sb[lhs][:, kk, :],
                                feats[:, kk, n * CH:(n + 1) * CH],
                                start=(kk == 0), stop=(kk == 1),
                            )
                        nc.scalar.activation(
                            tgt[:, n * CH:(n + 1) * CH], ps[0:rows, :],
                            AF.Identity, bias=sb[bias_nm][:, 0:1],
                        )
                # absorb the first zin-chunk ACT waits so the t=0 injects
                # carry a single semaphore wait
                zin_anchor = None
                for j, tgt in enumerate((zinY, zinAO, g0in)):
                    a2 = tgt[0:1, NF - 1:NF]   # last chunk -> max ACT sem value
                    zin_anchor = nc.tensor.matmul(
                        absorb[:, len(srcs) + j:len(srcs) + j + 1],
                        a2, a2, start=True, stop=True)

            # ---- Phase B: the scan ----
            with tc.tile_pool(name="st", bufs=1) as stp, \
                 tc.tile_pool(name="sc", bufs=3) as scp, \
                 tc.tile_pool(name="pY", bufs=1, space="PSUM") as pY, \
                 tc.tile_pool(name="pAO", bufs=1, space="PSUM") as pAO, \
                 tc.tile_pool(name="pC", bufs=1, space="PSUM") as pC, \
                 tc.tile_pool(name="p0", bufs=1, space="PSUM") as p0p, \
                 tc.tile_pool(name="p1", bufs=1, space="PSUM") as p1p, \
                 tc.tile_pool(name="p2", bufs=1, space="PSUM") as p2p, \
                 tc.tile_pool(name="pDS", bufs=1, space="PSUM") as pDS:

                # ping-pong state tiles; rows outside the written blocks are
                # memset to zero once and never rewritten
                xA = stp.tile([97, BS], F32, tag="xA")
                xB = stp.tile([97, BS], F32, tag="xB")
                f0A = stp.tile([96 + INTER, BS], F32, tag="f0A")
                f0B = stp.tile([96 + INTER, BS], F32, tag="f0B")
                f1A = stp.tile([96 + COMMAND, BS], F32, tag="f1A")
                f1B = stp.tile([96 + COMMAND, BS], F32, tag="f1B")
                f2A = stp.tile([96 + MOTOR, BS], F32, tag="f2A")
                f2B = stp.tile([96 + MOTOR, BS], F32, tag="f2B")
                for t_ in (xA, xB, f0A, f0B, f1A, f1B, f2A, f2B):
                    nc.vector.memset(t_, 0.0)
                x_pair = (xA, xB)
                f_pairs = ((f0A, f0B), (f1A, f1B), (f2A, f2B))
                prev_anchor = zin_anchor  # last Cd matmul of the previous step

                scan_reps = 1
                for o in opts:
                    if isinstance(o, str) and o.startswith("reps"):
                        scan_reps = int(o[4:])
                steps = [(rep, t) for rep in range(scan_reps) for t in range(T)]
                for rep, t in steps:
                    x_cur = x_pair[t % 2]
                    x_next = x_pair[(t + 1) % 2]
                    F0p, F1p, F2p = (fp[(t + 1) % 2] for fp in f_pairs)   # prev step
                    F0, F1, F2 = (fp[t % 2] for fp in f_pairs)            # this step
                    c0, c1 = t * BS, (t + 1) * BS

                    # --- LSTM gates: zin inject + recurrent part contracted
                    #     against the previous step's F tiles (h never formed) ---
                    PY = pY.tile([97, BS], F32)
                    PA = pAO.tile([97, BS], F32)
                    last = (t == 0)
                    miY = nc.tensor.matmul(PY, sb["I97"], zinY[:, c0:c1], start=True, stop=last)
                    miA = nc.tensor.matmul(PA, sb["I97"], zinAO[:, c0:c1], start=True, stop=last)
                    # keep injects behind the previous step's last Cd matmul in
                    # PE order so their WAR-on-ACT wait is already subsumed
                    # (the fused LDWEIGHTS can carry only one sem wait)
                    if prev_anchor is not None:
                        add_dep_helper(miY.ins, prev_anchor.ins, sync=False,
                                       reason="inject after prev Cd (wait budget)")
                        add_dep_helper(miA.ins, prev_anchor.ins, sync=False,
                                       reason="inject after prev Cd (wait budget)")
                    if t > 0:
                        for i, (Fprev, kb) in enumerate(((F0p, INTER), (F1p, COMMAND), (F2p, MOTOR))):
                            lastb = (i == 2)
                            nc.tensor.matmul(PY, sb[f"whY{i}"], Fprev[:, :],
                                             start=False, stop=lastb)
                            nc.tensor.matmul(PA, sb[f"whAO{i}"], Fprev[:, :],
                                             start=False, stop=lastb)

                    Y = scp.tile([97, BS], F32, tag="Y")
                    nc.scalar.activation(Y, PY, AF.Sigmoid)               # sig(fg)|sig(ig)
                    nc.scalar.activation(x_cur[64:97, :], PA[64:97, :], AF.Tanh)  # tanh(ia)
                    O = scp.tile([33, BS], F32, tag="O")
                    nc.scalar.activation(O, PA[0:33, :], AF.Sigmoid)      # sig(og)

                    S = scp.tile([97, BS], F32, tag="S")
                    nc.vector.tensor_mul(S, x_cur, Y)                     # c*sfg | T_ia*sig
                    PCt = pC.tile([33, BS], F32)
                    nc.tensor.matmul(PCt, sb["Cc"], S, start=True, stop=True)  # c_new
                    Tc = scp.tile([33, BS], F32, tag="Tc")
                    nc.scalar.activation(Tc, PCt, AF.Tanh)
                    nc.scalar.copy(x_next[0:33, :], PCt)                  # carry c
                    hl = scp.tile([33, BS], F32, tag="hl")
                    nc.vector.tensor_mul(hl, Tc, O)                       # h_lstm

                    # --- CfC layer 0 ---
                    P0 = p0p.tile([82, BS], F32)
                    mi0 = nc.tensor.matmul(P0, sb["I82"], g0in[:, c0:c1], start=True, stop=False)
                    if prev_anchor is not None:
                        add_dep_helper(mi0.ins, prev_anchor.ins, sync=False,
                                       reason="inject after prev Cd (wait budget)")
                    nc.tensor.matmul(P0, sb["W0recT"], hl[0:18, :], start=False, stop=True)
                    nc.scalar.activation(F0[0:82, :], P0, AF.Tanh)
                    D0 = pDS.tile([INTER, BS], F32, tag="DSd")
                    nc.tensor.matmul(D0, sb["Cd0"], F0[0:32 + INTER, :], start=True, stop=True)
                    nc.vector.tensor_mul(F0[96:96 + INTER, :], F0[64:64 + INTER, :], D0)

                    # --- CfC layer 1 ---
                    P1 = p1p.tile([76, BS], F32)
                    nc.tensor.matmul(P1, sb["W1recT"], hl[0:33, :], start=True, stop=False)
                    nc.tensor.matmul(P1, sb["W1comb"], F0[:, :], start=False, stop=True)
                    nc.scalar.activation(F1[0:76, :], P1, AF.Tanh, bias=sb["bias1"][:, 0:1])
                    D1 = pDS.tile([COMMAND, BS], F32, tag="DSd")
                    nc.tensor.matmul(D1, sb["Cd1"], F1[0:32 + COMMAND, :], start=True, stop=True)
                    nc.vector.tensor_mul(F1[96:96 + COMMAND, :], F1[64:64 + COMMAND, :], D1)

                    # --- CfC layer 2 ---
                    P2 = p2p.tile([67, BS], F32)
                    nc.tensor.matmul(P2, sb["W2recT"], hl[0:33, :], start=True, stop=False)
                    nc.tensor.matmul(P2, sb["W2comb"], F1[:, :], start=False, stop=True)
                    nc.scalar.activation(F2[0:67, :], P2, AF.Tanh, bias=sb["bias2"][:, 0:1])
                    D2 = pDS.tile([MOTOR, BS], F32, tag="DSd")
                    prev_anchor = nc.tensor.matmul(
                        D2, sb["Cd2"], F2[0:32 + MOTOR, :], start=True, stop=True)
                    nc.vector.tensor_mul(F2[96:96 + MOTOR, :], F2[64:64 + MOTOR, :], D2)
                    # motor output hl2 = s2 + 0.5*pt2 (off the critical chain)
                    DS2 = pDS.tile([MOTOR, BS], F32, tag="DS")
                    nc.tensor.matmul(DS2, sb["C2"], F2[0:32 + MOTOR, :], start=True, stop=True)
                    nc.vector.scalar_tensor_tensor(
                        out_sb[:, c0:c1], F2[96:96 + MOTOR, :], 0.5, DS2,
                        mybir.AluOpType.mult, mybir.AluOpType.add)

            nc.sync.dma_start(out=out_d[:, :], in_=out_sb[:, :])
    nc.compile()   # bacc passes: split multi-waits into event semaphores etc.
    return nc


def host_prep(inputs, T=T_FULL):
    """Shard + transpose x per core; fold weights (shared)."""
    x = np.asarray(inputs["x"], np.float32)
    w = prep_weights(inputs)
    in_maps = []
    for i in range(N_CORES):
        xs = x[i * BS:(i + 1) * BS, :T, :]                  # (BS, T, 512)
        xt = np.ascontiguousarray(xs.transpose(2, 1, 0).reshape(IN_DIM, T * BS))
        m = {"xt": xt}
        m.update(w)
        in_maps.append(m)
    return in_maps


def gather_output(results, T=T_FULL):
    outs = []
    for i in range(N_CORES):
        o = np.asarray(results[i]["out"])                   # (3, T*BS)
        outs.append(o.reshape(MOTOR, T, BS).transpose(2, 1, 0))  # (BS, T, 3)
    return np.concatenate(outs, axis=0)


_PROGRAM_CACHE = {}


def kernel(**inputs):
    T = T_FULL
    if T not in _PROGRAM_CACHE:
        _PROGRAM_CACHE[T] = build_program(T)
    nc = _PROGRAM_CACHE[T]
    in_maps = host_prep(inputs, T)
    res = run_bass_kernel_spmd(nc, in_maps, list(range(N_CORES)))
    return gather_output(res.results, T)


# revision 53
# speedup vs baseline: 2.0669x; 1.0512x over previous
"""Trainium2 Bass kernel for the DinMod LSTM+CfC (NCP) recurrent network.

Strategy:
  - Data-parallel over 8 NeuronCores: batch 64 -> 8 per core, weights replicated.
  - Phase A (parallel): fc1 projection feats = x @ fc1_w.T + b, then the
    time-invariant input projections of the LSTM cell and CfC layer 0 are
    precomputed for all T steps as big matmuls (transposed layout: feature
    dim on partitions, (t, b) on the free dim).
  - Phase B (sequential scan over T=512): tiny per-step recurrent cell with a
    minimized critical path:
      * precomputed per-step terms are injected into PSUM via identity
        matmuls (off the critical path) and the recurrent matmuls accumulate
        on top (start=False),
      * sigmoid for the CfC gate is 0.5*tanh(0.5x)+0.5 with the 0.5 folded
        into weights, so each CfC layer needs a single Tanh activation op,
      * elementwise gate algebra is packed into quadrant-aligned "stacked"
        tiles so one vector op covers two gates,
      * the CfC state hl = 0.5*(f1 + f2 + t*(f2-f1)) is never materialized:
        t*(f2-f1) is written into spare rows of the F tile and every
        consumer contracts F directly with host-precomposed weights.
  - All operand blocks live at SBUF partition bases in {0, 32, 64, 96}
    (quadrant addressing); dual-SBUF vector ops use equal bases, f2-f1 is
    built on the tensor engine (PSUM) so the p-multiply is mixed-space.
  - Every compute instruction carries at most one new semaphore wait
    (hardware limit): absorber 1x1 matmuls observe each DMA once, injects
    are pinned behind the previous step's Cd matmul, and bacc's
    generate_event_semaphores splits anything left.

Layout conventions (partition ranges):
  P_Y/Y (97p):  sig(fg) 0:33 | sig(ig) 64:97
  P_AO (97p):   og 0:33 | ia 64:97
  X (97p):      c 0:33  | tanh(ia) 64:97
  P_l/F_l (CfC layer l, k outputs): f1 0:k | f2 32:32+k | t 64:64+k | pt 96:96+k
"""

import numpy as np

import concourse.bass as bass
import concourse.mybir as mybir
from concourse import bacc
from concourse.tile import TileContext
from concourse.tile_rust import add_dep_helper
from concourse.bass_utils import run_bass_kernel_spmd

IN_DIM, LATENT = 512, 256
INTER, COMMAND, MOTOR = 18, 12, 3
STATE = INTER + COMMAND + MOTOR  # 33
B, T_FULL, N_CORES = 64, 512, 8
BS = B // N_CORES  # 8

F32 = mybir.dt.float32
AF = mybir.ActivationFunctionType


def _gpos(j):
    """state index (0..32) -> gapped partition position."""
    if j < INTER:
        return j
    if j < INTER + COMMAND:
        return 32 + (j - INTER)
    return 64 + (j - INTER - COMMAND)


def prep_weights(inp):
    """Fold/transpose all model weights into device layouts. numpy f32."""
    g = {k: np.asarray(v, np.float32) for k, v in inp.items()}
    w = {}
    w["fc1T"] = np.ascontiguousarray(g["fc1_w"].T)             # (512, 256)
    fb = np.zeros((128, 2), np.float32)
    fb[:, 0] = g["fc1_b"][:128]
    fb[:, 1] = g["fc1_b"][128:]
    w["fc1b"] = fb

    wi, bi, wh = g["lstm_wi"], g["lstm_bi"], g["lstm_wh"]
    ia, ig, fg, og = (slice(0, 33), slice(33, 66), slice(66, 99), slice(99, 132))

    def pack97(rows_lo, rows_hi, src, axis_cols):
        """Build (axis_cols, 97) matrix: cols 0:33 <- src[rows_lo], 64:97 <- src[rows_hi]."""
        m = np.zeros((axis_cols, 97), np.float32)
        m[:, 0:33] = src[rows_lo].T
        m[:, 64:97] = src[rows_hi].T
        return m

    w["wiT_Y"] = pack97(fg, ig, wi, LATENT)                     # (256, 97)
    w["wiT_AO"] = pack97(og, ia, wi, LATENT)
    bY = np.zeros((97, 1), np.float32)
    bY[0:33, 0] = bi[fg] + 1.0
    bY[64:97, 0] = bi[ig]
    w["biasY"] = bY
    bAO = np.zeros((97, 1), np.float32)
    bAO[0:33, 0] = bi[og]
    bAO[64:97, 0] = bi[ia]
    w["biasAO"] = bAO

    # LSTM recurrent weights.  h_state = concat(hl_0, hl_1, hl_2) and each
    # hl_l = s_l + p_l with s_l = Ms_l @ [f1;f2].  We never materialize h:
    # consumers contract directly against [F_l (with p packed at rows 96:)],
    # with lhsT rows 0:2k = Ms_l.T @ W_block, rows 96:96+k = W_block.
    # hl_l = 0.5*(f1 + f2 + pt_l) where pt_l = t*(f2-f1) sits at rows 96: of
    # the F_l tile; consumers contract F_l directly with [0.5W; 0.5W; 0.5W].
    ks = [INTER, COMMAND, MOTOR]
    koff = [0, INTER, INTER + COMMAND]
    for bi_, k in enumerate(ks):
        for pname, lo_sl, hi_sl in (("Y", fg, ig), ("AO", og, ia)):
            blk = np.zeros((k, 97), np.float32)   # raw W rows for this state block
            for j in range(k):
                blk[j, 0:33] = wh[lo_sl, koff[bi_] + j]
                blk[j, 64:97] = wh[hi_sl, koff[bi_] + j]
            comb = np.zeros((96 + k, 97), np.float32)
            comb[0:k, :] = 0.5 * blk
            comb[32:32 + k, :] = 0.5 * blk
            comb[96:96 + k, :] = 0.5 * blk
            w[f"wh{pname}{bi_}"] = comb

    # CfC layers
    dims = [(LATENT, INTER), (INTER, COMMAND), (COMMAND, MOTOR)]
    for l, (p_l, k) in enumerate(dims):
        w1m = g[f"ff1w{l}"] * g[f"mask{l}"]
        w2m = g[f"ff2w{l}"] * g[f"mask{l}"]
        wab = 0.5 * (g[f"taw{l}"] + g[f"tbw{l}"])
        bti = 0.5 * (g[f"tab{l}"] + g[f"tbb{l}"])
        nrows = 64 + k  # gate rows: f1 0:k | f2 32:32+k | t 64:64+k
        bias = np.zeros((nrows, 1), np.float32)
        bias[0:k, 0] = g[f"ff1b{l}"]
        bias[32:32 + k, 0] = g[f"ff2b{l}"]
        bias[64:64 + k, 0] = bti
        w[f"bias{l}"] = bias

        def pack_cols(col_sel, in_rows, row_map):
            # gate-row layout (quadrant per role): f1 0:k | f2 32:32+k | t 64:64+k
            m = np.zeros((in_rows, nrows), np.float32)
            for jj, rr in row_map:
                m[rr, 0:k] = w1m[:, col_sel][:, jj]
                m[rr, 32:32 + k] = w2m[:, col_sel][:, jj]
                m[rr, 64:64 + k] = wab[:, col_sel][:, jj]
            return m

        if l == 0:
            # input part (from feats, 256) and recurrent part (inter slice)
            w["W0inT"] = pack_cols(slice(0, LATENT), LATENT,
                                   [(jj, jj) for jj in range(LATENT)])
            w["W0recT"] = pack_cols(slice(LATENT, LATENT + k), INTER,
                                    [(jj, jj) for jj in range(INTER)])
        elif l == 1:
            # input = hl0 = 0.5*(f1 + f2 + pt0), contracted against F0ext
            w1in = pack_cols(slice(0, INTER), INTER,
                             [(jj, jj) for jj in range(INTER)])      # (18, 76)
            comb = np.zeros((96 + INTER, nrows), np.float32)
            comb[0:INTER, :] = 0.5 * w1in
            comb[32:32 + INTER, :] = 0.5 * w1in
            comb[96:96 + INTER, :] = 0.5 * w1in
            w["W1comb"] = comb
            w["W1recT"] = pack_cols(slice(INTER, INTER + k), STATE,
                                    [(jj, INTER + jj) for jj in range(COMMAND)])
        else:
            # input = hl1 = 0.5*(f1 + f2 + pt1), contracted against F1ext
            w2in = pack_cols(slice(0, COMMAND), COMMAND,
                             [(jj, jj) for jj in range(COMMAND)])    # (12, 67)
            comb = np.zeros((96 + COMMAND, nrows), np.float32)
            comb[0:COMMAND, :] = 0.5 * w2in
            comb[32:32 + COMMAND, :] = 0.5 * w2in
            comb[96:96 + COMMAND, :] = 0.5 * w2in
            w["W2comb"] = comb
            w["W2recT"] = pack_cols(slice(COMMAND, COMMAND + k), STATE,
                                    [(jj, STATE - MOTOR + jj) for jj in range(MOTOR)])

    bg0 = np.zeros((64 + INTER, 1), np.float32)
    bg0[0:INTER, 0] = g["ff1b0"]
    bg0[32:32 + INTER, 0] = g["ff2b0"]
    bg0[64:64 + INTER, 0] = 0.5 * (g["tab0"] + g["tbb0"])
    w["biasg0"] = bg0

    # constant matrices for tensor-engine linear combos
    cc = np.zeros((97, 33), np.float32)
    for j in range(33):
        cc[j, j] = 1.0
        cc[64 + j, j] = 1.0
    w["Cc"] = cc
    # s2 = 0.5*(f1 + f2) for the motor output (hl2 = s2 + 0.5*pt2)
    c = np.zeros((32 + MOTOR, MOTOR), np.float32)
    for j in range(MOTOR):
        c[j, j] = 0.5
        c[32 + j, j] = 0.5
    w["C2"] = c
    # d_l = f2 - f1 on the tensor engine (PSUM out, so the p-multiply reads
    # one PSUM operand — dual-SBUF TensorTensor requires equal base partitions)
    for l, k in [(0, INTER), (1, COMMAND), (2, MOTOR)]:
        c = np.zeros((32 + k, k), np.float32)
        for j in range(k):
            c[j, j] = -1.0
            c[32 + j, j] = 1.0
        w[f"Cd{l}"] = c
    i97 = np.zeros((97, 97), np.float32)
    for r in list(range(33)) + list(range(64, 97)):
        i97[r, r] = 1.0
    w["I97"] = i97
    i82 = np.zeros((82, 82), np.float32)
    for r in (list(range(INTER)) + list(range(32, 32 + INTER))
              + list(range(64, 64 + INTER))):
        i82[r, r] = 1.0
    w["I82"] = i82
    return w


# DRAM input specs (name -> shape) besides xt
def _weight_specs(T):
    return {
        "fc1T": (512, 256), "fc1b": (128, 2),
        "wiT_Y": (256, 97), "wiT_AO": (256, 97),
        "biasY": (97, 1), "biasAO": (97, 1),
        "whY0": (114, 97), "whY1": (108, 97), "whY2": (99, 97),
        "whAO0": (114, 97), "whAO1": (108, 97), "whAO2": (99, 97),
        "W0inT": (256, 82), "W0recT": (18, 82), "biasg0": (82, 1),
        "W1comb": (114, 76), "W1recT": (33, 76), "bias1": (76, 1),
        "W2comb": (108, 67), "W2recT": (33, 67), "bias2": (67, 1),
        "Cc": (97, 33), "C2": (35, 3),
        "Cd0": (50, 18), "Cd1": (44, 12), "Cd2": (35, 3),
        "I97": (97, 97), "I82": (82, 82),
    }


def build_program(T=T_FULL, opts=()):
    """Build the Bass program for one core: xt (512, T*BS) -> out (3, T*BS)."""
    opts = set(opts)
    NF = T * BS
    nc = bacc.Bacc("TRN2")
    xt_d = nc.dram_tensor("xt", [IN_DIM, NF], F32, kind="ExternalInput")
    wd = {}
    for nm, shp in _weight_specs(T).items():
        wd[nm] = nc.dram_tensor(nm, list(shp), F32, kind="ExternalInput")
    out_d = nc.dram_tensor("out", [MOTOR, NF], F32, kind="ExternalOutput")

    NCH = NF // 512 if NF >= 512 else 1   # free-dim chunks for phase A
    CH = min(512, NF)

    with TileContext(nc) as tc:
        with tc.tile_pool(name="wpool", bufs=1) as wp, \
             tc.tile_pool(name="data", bufs=1) as dp:
            # ---- load weights (tensors with >128 rows are partition-chunked) ----
            sb = {}
            for nm, shp in _weight_specs(T).items():
                rows, cols = shp
                if rows > 128:
                    nch = (rows + 127) // 128
                    t = wp.tile([128, nch, cols], F32, tag=f"w_{nm}")
                    # single DMA per tensor so consumers wait on one queue only
                    nc.sync.dma_start(
                        out=t, in_=wd[nm].rearrange("(c p) n -> p c n", p=128))
                else:
                    t = wp.tile([rows, cols], F32, tag=f"w_{nm}")
                    nc.sync.dma_start(out=t, in_=wd[nm][:, :])
                sb[nm] = t

            # ---- load x (transposed on host): 4 chunks of 128 partitions ----
            xt_sb = dp.tile([128, 4, NF], F32)
            nc.sync.dma_start(out=xt_sb,
                              in_=xt_d.rearrange("(c p) n -> p c n", p=128))

            feats = dp.tile([128, 2, NF], F32)
            zinY = dp.tile([97, NF], F32)
            zinAO = dp.tile([97, NF], F32)
            g0in = dp.tile([82, NF], F32)
            out_sb = dp.tile([MOTOR, NF], F32)

            # ---- Phase A: big parallel matmuls ----
            with tc.tile_pool(name="pa", bufs=2, space="PSUM") as pa:
                # Wait-absorbers: the fused LDWEIGHTS+MATMUL can carry only one
                # semaphore wait, so have the PE observe every DMA-loaded tile
                # once via a 1x1 dummy matmul (one producer each, disjoint
                # PSUM columns so no write-ordering self-wait is added);
                # real matmuls then need at most one new wait.
                srcs = list(sb.values()) + [xt_sb]
                absorb = pa.tile([1, len(srcs) + 3], F32, tag="absorb")
                for j, t_ in enumerate(srcs):
                    a2 = t_[0:1, 0, 0:1] if len(t_.shape) == 3 else t_[0:1, 0:1]
                    nc.tensor.matmul(absorb[:, j:j + 1], a2, a2, start=True, stop=True)
                for m in range(2):
                    for n in range(NCH):
                        ps = pa.tile([128, CH], F32)
                        for k in range(4):
                            nc.tensor.matmul(
                                ps,
                                sb["fc1T"][:, k, 128 * m:128 * (m + 1)],
                                xt_sb[:, k, n * CH:(n + 1) * CH],
                                start=(k == 0), stop=(k == 3),
                            )
                        nc.scalar.activation(
                            feats[:, m, n * CH:(n + 1) * CH], ps,
                            AF.Identity, bias=sb["fc1b"][:, m:m + 1],
                        )
                for tgt, lhs, bias_nm, rows in (
                    (zinY, "wiT_Y", "biasY", 97),
                    (zinAO, "wiT_AO", "biasAO", 97),
                    (g0in, "W0inT", "biasg0", 82),
                ):
                    for n in range(NCH):
                        ps = pa.tile([128, CH], F32)
                        for kk in range(2):
                            nc.tensor.matmul(
                                ps[0:rows, :],
                                sb[lhs][:, kk, :],
                                feats[:, kk, n * CH:(n + 1) * CH],
                                start=(kk == 0), stop=(kk == 1),
                            )
                        nc.scalar.activation(
                            tgt[:, n * CH:(n + 1) * CH], ps[0:rows, :],
                            AF.Identity, bias=sb[bias_nm][:, 0:1],
                        )
                # absorb the first zin-chunk ACT waits so the t=0 injects
                # carry a single semaphore wait
                zin_anchor = None
                for j, tgt in enumerate((zinY, zinAO, g0in)):
                    a2 = tgt[0:1, NF - 1:NF]   # last chunk -> max ACT sem value
                    zin_anchor = nc.tensor.matmul(
                        absorb[:, len(srcs) + j:len(srcs) + j + 1],
                        a2, a2, start=True, stop=True)

            # ---- Phase B: the scan ----
            with tc.tile_pool(name="st", bufs=1) as stp, \
                 tc.tile_pool(name="sc", bufs=3) as scp, \
                 tc.tile_pool(name="pY", bufs=1, space="PSUM") as pY, \
                 tc.tile_pool(name="pAO", bufs=1, space="PSUM") as pAO, \
                 tc.tile_pool(name="pC", bufs=1, space="PSUM") as pC, \
                 tc.tile_pool(name="p0", bufs=1, space="PSUM") as p0p, \
                 tc.tile_pool(name="p1", bufs=1, space="PSUM") as p1p, \
                 tc.tile_pool(name="p2", bufs=1, space="PSUM") as p2p, \
                 tc.tile_pool(name="pDS", bufs=1, space="PSUM") as pDS:

                # ping-pong state tiles; rows outside the written blocks are
                # memset to zero once and never rewritten
                xA = stp.tile([97, BS], F32, tag="xA")
                xB = stp.tile([97, BS], F32, tag="xB")
                f0A = stp.tile([96 + INTER, BS], F32, tag="f0A")
                f0B = stp.tile([96 + INTER, BS], F32, tag="f0B")
                f1A = stp.tile([96 + COMMAND, BS], F32, tag="f1A")
                f1B = stp.tile([96 + COMMAND, BS], F32, tag="f1B")
                f2A = stp.tile([96 + MOTOR, BS], F32, tag="f2A")
                f2B = stp.tile([96 + MOTOR, BS], F32, tag="f2B")
                for t_ in (xA, xB, f0A, f0B, f1A, f1B, f2A, f2B):
                    nc.vector.memset(t_, 0.0)
                x_pair = (xA, xB)
                f_pairs = ((f0A, f0B), (f1A, f1B), (f2A, f2B))
                prev_anchor = zin_anchor  # last Cd matmul of the previous step

                scan_reps = 1
                for o in opts:
                    if isinstance(o, str) and o.startswith("reps"):
                        scan_reps = int(o[4:])
                steps = [(rep, t) for rep in range(scan_reps) for t in range(T)]
                for rep, t in steps:
                    x_cur = x_pair[t % 2]
                    x_next = x_pair[(t + 1) % 2]
                    F0p, F1p, F2p = (fp[(t + 1) % 2] for fp in f_pairs)   # prev step
                    F0, F1, F2 = (fp[t % 2] for fp in f_pairs)            # this step
                    c0, c1 = t * BS, (t + 1) * BS

                    # --- LSTM gates: zin inject + recurrent part contracted
                    #     against the previous step's F tiles (h never formed) ---
                    PY = pY.tile([97, BS], F32)
                    PA = pAO.tile([97, BS], F32)
                    last = (t == 0)
                    miY = nc.tensor.matmul(PY, sb["I97"], zinY[:, c0:c1], start=True, stop=last)
                    miA = nc.tensor.matmul(PA, sb["I97"], zinAO[:, c0:c1], start=True, stop=last)
                    # keep injects behind the previous step's last Cd matmul in
                    # PE order so their WAR-on-ACT wait is already subsumed
                    # (the fused LDWEIGHTS can carry only one sem wait)
                    if prev_anchor is not None:
                        add_dep_helper(miY.ins, prev_anchor.ins, sync=False,
                                       reason="inject after prev Cd (wait budget)")
                        add_dep_helper(miA.ins, prev_anchor.ins, sync=False,
                                       reason="inject after prev Cd (wait budget)")
                    if t > 0:
                        for i, (Fprev, kb) in enumerate(((F0p, INTER), (F1p, COMMAND), (F2p, MOTOR))):
                            lastb = (i == 2)
                            nc.tensor.matmul(PY, sb[f"whY{i}"], Fprev[:, :],
                                             start=False, stop=lastb)
                            nc.tensor.matmul(PA, sb[f"whAO{i}"], Fprev[:, :],
                                             start=False, stop=lastb)

                    Y = scp.tile([97, BS], F32, tag="Y")
                    nc.scalar.activation(Y, PY, AF.Sigmoid)               # sig(fg)|sig(ig)
                    nc.scalar.activation(x_cur[64:97, :], PA[64:97, :], AF.Tanh)  # tanh(ia)
                    O = scp.tile([33, BS], F32, tag="O")
                    nc.scalar.activation(O, PA[0:33, :], AF.Sigmoid)      # sig(og)

                    S = scp.tile([97, BS], F32, tag="S")
                    nc.vector.tensor_mul(S, x_cur, Y)                     # c*sfg | T_ia*sig
                    PCt = pC.tile([33, BS], F32)
                    nc.tensor.matmul(PCt, sb["Cc"], S, start=True, stop=True)  # c_new
                    Tc = scp.tile([33, BS], F32, tag="Tc")
                    nc.scalar.activation(Tc, PCt, AF.Tanh)
                    hl = scp.tile([33, BS], F32, tag="hl")
                    nc.vector.tensor_mul(hl, Tc, O)                       # h_lstm

                    # --- CfC layer 0 ---
                    P0 = p0p.tile([82, BS], F32)
                    mi0 = nc.tensor.matmul(P0, sb["I82"], g0in[:, c0:c1], start=True, stop=False)
                    if prev_anchor is not None:
                        add_dep_helper(mi0.ins, prev_anchor.ins, sync=False,
                                       reason="inject after prev Cd (wait budget)")
                    nc.tensor.matmul(P0, sb["W0recT"], hl[0:18, :], start=False, stop=True)
                    nc.scalar.activation(F0[0:82, :], P0, AF.Tanh)
                    # carry c for the next step: emitted after F0 so the copy
                    # lands in the ScalarE idle window instead of delaying F0
                    # (it is only needed by the next step's S-multiply)
                    nc.scalar.copy(x_next[0:33, :], PCt)
                    D0 = pDS.tile([INTER, BS], F32, tag="DSd")
                    nc.tensor.matmul(D0, sb["Cd0"], F0[0:32 + INTER, :], start=True, stop=True)
                    nc.vector.tensor_mul(F0[96:96 + INTER, :], F0[64:64 + INTER, :], D0)

                    # --- CfC layer 1 ---
                    P1 = p1p.tile([76, BS], F32)
                    nc.tensor.matmul(P1, sb["W1recT"], hl[0:33, :], start=True, stop=False)
                    nc.tensor.matmul(P1, sb["W1comb"], F0[:, :], start=False, stop=True)
                    nc.scalar.activation(F1[0:76, :], P1, AF.Tanh, bias=sb["bias1"][:, 0:1])
                    D1 = pDS.tile([COMMAND, BS], F32, tag="DSd")
                    nc.tensor.matmul(D1, sb["Cd1"], F1[0:32 + COMMAND, :], start=True, stop=True)
                    nc.vector.tensor_mul(F1[96:96 + COMMAND, :], F1[64:64 + COMMAND, :], D1)

                    # --- CfC layer 2 ---
                    P2 = p2p.tile([67, BS], F32)
                    nc.tensor.matmul(P2, sb["W2recT"], hl[0:33, :], start=True, stop=False)
                    nc.tensor.matmul(P2, sb["W2comb"], F1[:, :], start=False, stop=True)
                    nc.scalar.activation(F2[0:67, :], P2, AF.Tanh, bias=sb["bias2"][:, 0:1])
                    D2 = pDS.tile([MOTOR, BS], F32, tag="DSd")
                    prev_anchor = nc.tensor.matmul(
                        D2, sb["Cd2"], F2[0:32 + MOTOR, :], start=True, stop=True)
                    nc.vector.tensor_mul(F2[96:96 + MOTOR, :], F2[64:64 + MOTOR, :], D2)
                    # motor output hl2 = s2 + 0.5*pt2 (off the critical chain)
                    DS2 = pDS.tile([MOTOR, BS], F32, tag="DS")
                    nc.tensor.matmul(DS2, sb["C2"], F2[0:32 + MOTOR, :], start=True, stop=True)
                    nc.vector.scalar_tensor_tensor(
                        out_sb[:, c0:c1], F2[96:96 + MOTOR, :], 0.5, DS2,
                        mybir.AluOpType.mult, mybir.AluOpType.add)

            nc.sync.dma_start(out=out_d[:, :], in_=out_sb[:, :])
    nc.compile()   # bacc passes: split multi-waits into event semaphores etc.
    return nc


def host_prep(inputs, T=T_FULL):
    """Shard + transpose x per core; fold weights (shared)."""
    x = np.asarray(inputs["x"], np.float32)
    w = prep_weights(inputs)
    in_maps = []
    for i in range(N_CORES):
        xs = x[i * BS:(i + 1) * BS, :T, :]                  # (BS, T, 512)
        xt = np.ascontiguousarray(xs.transpose(2, 1, 0).reshape(IN_DIM, T * BS))
        m = {"xt": xt}
        m.update(w)
        in_maps.append(m)
    return in_maps


def gather_output(results, T=T_FULL):
    outs = []
    for i in range(N_CORES):
        o = np.asarray(results[i]["out"])                   # (3, T*BS)
        outs.append(o.reshape(MOTOR, T, BS).transpose(2, 1, 0))  # (BS, T, 3)
    return np.concatenate(outs, axis=0)


_PROGRAM_CACHE = {}


def kernel(**inputs):
    T = T_FULL
    if T not in _PROGRAM_CACHE:
        _PROGRAM_CACHE[T] = build_program(T)
    nc = _PROGRAM_CACHE[T]
    in_maps = host_prep(inputs, T)
    res = run_bass_kernel_spmd(nc, in_maps, list(range(N_CORES)))
    return gather_output(res.results, T)
